# revision 12
# baseline (speedup 1.0000x reference)
"""Distributed MQA attention block (N=2, L=1024, D=4544, H=71, Dkv=64) on 8 TRN2 cores.

Sharding: 2 batch groups x 4-way head tensor-parallel.
  core c = 4*g + j: batch g, heads [18j, 18j+18) (core j=3: 17 real heads + 1 zero pad).
Per core: QKV projection (bf16), RoPE (rotation via PE matmul), causal attention in
S^T = K@Q^T orientation (softmax sum fused into the V-matmul via an appended
ones-column), AllGather of attn^T (bf16) within each 4-core group per q-half,
column-sharded dense projection. Host casts inputs to bf16 / pre-transposes, and
assembles the 8 [1136, 1024] f32 output shards.
"""

import sys

if "/opt/trn_rl_repo" not in sys.path:
    sys.path.insert(0, "/opt/trn_rl_repo")

import numpy as np
import ml_dtypes

import concourse.bass as bass
import concourse.bacc as bacc
import concourse.mybir as mybir
import concourse.tile as tile
from concourse.bass_utils import run_bass_kernel_spmd

BF16 = mybir.dt.bfloat16
F32 = mybir.dt.float32
AF = mybir.ActivationFunctionType

N, L, D = 2, 1024, 4544
H, DKV = 71, 64
NCORES, GSZ = 8, 4
HPC = 18                 # heads per core (last core of each group: 17 real + 1 pad)
DLOC = HPC * DKV         # 1152
DPAD = GSZ * DLOC        # 4608 = 36 * 128
ESH = D // GSZ           # 1136 output-column shard
NET = 36                 # e-contraction tiles over D=4544 (35 x 128 + 1 x 64)
NMT = DLOC // 128        # 9 m-tiles of Q^T rows (2 heads each)
QB = 512                 # q-block (half of L)
SCALE = 1.0 / np.sqrt(DKV)
REPLICA_GROUPS = [[0, 1, 2, 3], [4, 5, 6, 7]]

_CACHE = {}


def _esz(i):
    return 128 if i < NET - 1 else 64


def _emit(tc, nc, io):
    xT, wqkvT, wdT, cosT, sinT, rot, eye, masks, ones64, out = (
        io["xT"], io["wqkvT"], io["wdT"], io["cosT"], io["sinT"], io["rot"],
        io["eye"], io["masks"], io["ones64"], io["out"],
    )

    # ---- persistent SBUF (live through the whole kernel) ----
    pers = tc.alloc_tile_pool(name="pers", bufs=1)
    qsb = pers.tile([128, NMT * 1024], BF16, tag="qsb")    # roped Q^T, 2 heads/tile
    ksb = pers.tile([128, 1024], BF16, tag="ksb")          # roped K^T, dup in both halves
    vsb = pers.tile([128, 8 * 65], BF16, tag="vsb")        # V [tok,64]+ones col, 8 chunks
    cossb = pers.tile([128, 1024], BF16, tag="cossb")
    sinsb = pers.tile([128, 1024], BF16, tag="sinsb")
    rotsb = pers.tile([128, 128], BF16, tag="rotsb")
    eyesb = pers.tile([128, 128], BF16, tag="eyesb")
    masksb = pers.tile([128, 4 * QB], BF16, tag="masksb")
    onesb = pers.tile([1, 64], BF16, tag="onesb")

    nc.sync.dma_start(cossb[:, :], cosT[:, :])
    nc.sync.dma_start(sinsb[:, :], sinT[:, :])
    nc.sync.dma_start(rotsb[:, :], rot[:, :])
    nc.sync.dma_start(eyesb[:, :], eye[:, :])
    nc.sync.dma_start(masksb[:, :], masks[:, :])
    nc.sync.dma_start(onesb[:, :], ones64[:, :])
    nc.vector.memset(vsb[:, :], 1.0)  # ones column survives the V copies below

    # ---- DRAM bounce buffers for the per-q-half AllGather ----
    dram = tc.alloc_tile_pool(name="dram", bufs=1, space="DRAM")
    ag_in = [dram.tile([DLOC, QB], BF16, tag=f"agin{q}", name=f"agin{q}")
             for q in range(2)]
    ag_out = [dram.tile([DPAD, QB], BF16, tag=f"agout{q}", name=f"agout{q}")
              for q in range(2)]

    # ================= Phase A: QKV projection + RoPE =================
    with (
        tc.tile_pool(name="xp", bufs=1) as xp,
        tc.tile_pool(name="wqp", bufs=2) as wqp,
        tc.tile_pool(name="ra", bufs=2) as ra,
        tc.tile_pool(name="rt", bufs=2) as rt,
        tc.tile_pool(name="vt", bufs=1) as vt,
        tc.tile_pool(name="ps2", bufs=3, space="PSUM") as ps2,
    ):
        xsb = xp.tile([128, NET * 1024], BF16, tag="xsb")
        for i in range(NET):
            p = _esz(i)
            nc.sync.dma_start(xsb[0:p, i * 1024:(i + 1) * 1024],
                              xT[i * 128:i * 128 + p, :])

        def load_w_cols(dst, c0, cw):
            # wqkvT[:, c0:c0+cw] -> dst [128, NET*cw] (tile i at cols i*cw)
            nc.sync.dma_start(
                dst[:, 0:(NET - 1) * cw].rearrange("p (t c) -> p t c", c=cw),
                wqkvT[0:(NET - 1) * 128, c0:c0 + cw].rearrange(
                    "(t p) c -> p t c", p=128),
            )
            nc.sync.dma_start(dst[0:64, (NET - 1) * cw:NET * cw],
                              wqkvT[(NET - 1) * 128:D, c0:c0 + cw])

        # ---- K/V projections (shared KV head) ----
        wkv = wqp.tile([128, NET * 128], BF16, tag="wm")
        load_w_cols(wkv, DLOC, 128)

        # K^T [64, 1024] (psum, 2 banks)
        kps = ps2.tile([128, 1024], F32, tag="ps2")
        for i in range(NET):
            p = _esz(i)
            for q in range(2):
                nc.tensor.matmul(
                    kps[0:64, q * QB:(q + 1) * QB],
                    lhsT=wkv[0:p, i * 128:i * 128 + 64],
                    rhs=xsb[0:p, i * 1024 + q * QB:i * 1024 + (q + 1) * QB],
                    start=(i == 0), stop=(i == NET - 1),
                )
        # rope K (rows 0:64), then duplicate into rows 64:128 via DMA
        kraw = ra.tile([128, 1024], BF16, tag="ra")
        nc.scalar.copy(kraw[0:64, :], kps[0:64, :])
        krot = ps2.tile([128, 1024], F32, tag="ps2")
        for q in range(2):
            nc.tensor.matmul(krot[0:64, q * QB:(q + 1) * QB],
                             lhsT=rotsb[0:64, 0:64],
                             rhs=kraw[0:64, q * QB:(q + 1) * QB],
                             start=True, stop=True)
        for q in range(2):
            s = slice(q * QB, (q + 1) * QB)
            t1 = rt.tile([128, QB], F32, tag="t1")
            t2 = rt.tile([128, QB], F32, tag="t2")
            nc.vector.tensor_mul(t1[0:64, :], kraw[0:64, s], cossb[0:64, s])
            nc.vector.tensor_mul(t2[0:64, :], krot[0:64, s], sinsb[0:64, s])
            nc.vector.tensor_add(ksb[0:64, s], t1[0:64, :], t2[0:64, :])
        nc.sync.dma_start(ksb[64:128, :], ksb[0:64, :])

        # V^T [64, 1024] then transpose to V [tok, 64] chunks in vsb
        vps = ps2.tile([128, 1024], F32, tag="ps2")
        for i in range(NET):
            p = _esz(i)
            for q in range(2):
                nc.tensor.matmul(
                    vps[0:64, q * QB:(q + 1) * QB],
                    lhsT=wkv[0:p, i * 128 + 64:i * 128 + 128],
                    rhs=xsb[0:p, i * 1024 + q * QB:i * 1024 + (q + 1) * QB],
                    start=(i == 0), stop=(i == NET - 1),
                )
        vtsb = vt.tile([64, 1024], BF16, tag="vtsb")
        nc.scalar.copy(vtsb[:, :], vps[0:64, :])
        for t8 in range(8):
            vtp = ps2.tile([128, 2048], BF16, tag="ps2")
            nc.tensor.transpose(vtp[0:128, 0:64],
                                vtsb[0:64, t8 * 128:(t8 + 1) * 128],
                                eyesb[0:64, 0:64])
            nc.scalar.copy(vsb[:, t8 * 65:t8 * 65 + 64], vtp[0:128, 0:64])

        # ---- Q projection + RoPE, per m-tile (2 heads each) ----
        for m in range(NMT):
            wqm = wqp.tile([128, NET * 128], BF16, tag="wm")
            load_w_cols(wqm, m * 128, 128)
            qps = ps2.tile([128, 1024], F32, tag="ps2")
            for i in range(NET):
                p = _esz(i)
                for q in range(2):
                    nc.tensor.matmul(
                        qps[:, q * QB:(q + 1) * QB],
                        lhsT=wqm[0:p, i * 128:(i + 1) * 128],
                        rhs=xsb[0:p, i * 1024 + q * QB:i * 1024 + (q + 1) * QB],
                        start=(i == 0), stop=(i == NET - 1),
                    )
            qraw = ra.tile([128, 1024], BF16, tag="ra")
            nc.scalar.copy(qraw[:, :], qps[:, :])
            qrot = ps2.tile([128, 1024], F32, tag="ps2")
            for q in range(2):
                nc.tensor.matmul(qrot[:, q * QB:(q + 1) * QB],
                                 lhsT=rotsb[:, :],
                                 rhs=qraw[:, q * QB:(q + 1) * QB],
                                 start=True, stop=True)
            for q in range(2):
                s = slice(q * QB, (q + 1) * QB)
                t1 = rt.tile([128, QB], F32, tag="t1")
                t2 = rt.tile([128, QB], F32, tag="t2")
                nc.vector.tensor_mul(t1[:, :], qraw[:, s], cossb[:, s])
                nc.vector.tensor_mul(t2[:, :], qrot[:, s], sinsb[:, s])
                nc.vector.tensor_add(qsb[:, m * 1024 + q * QB:m * 1024 + (q + 1) * QB],
                                     t1[:, :], t2[:, :])

    # ================= Phase B: attention + AllGather + dense =================
    with (
        tc.tile_pool(name="ex", bufs=6) as ex,
        tc.tile_pool(name="at", bufs=4) as at,
        tc.tile_pool(name="rb", bufs=2) as rb,
        tc.tile_pool(name="rp", bufs=2) as rp,
        tc.tile_pool(name="wdp", bufs=1) as wdp,
        tc.tile_pool(name="gp", bufs=1) as gp,
        tc.tile_pool(name="op", bufs=3) as op,
        tc.tile_pool(name="sc", bufs=3, space="PSUM") as sc,
        tc.tile_pool(name="ac", bufs=3, space="PSUM") as ac,
        tc.tile_pool(name="dp", bufs=2, space="PSUM") as dp,
    ):
        # dense weights resident; DMAs run in the background during attention
        wdsb = wdp.tile([128, NET * ESH], BF16, tag="wdsb")
        for i in range(NET):
            nc.sync.dma_start(wdsb[:, i * ESH:(i + 1) * ESH],
                              wdT[i * 128:(i + 1) * 128, :])

        for qh in range(2):
            nkt = 4 * qh + 4
            for hp in range(NMT):
                accs = []
                for par in range(2):  # even head (rows 0:64), odd head (rows 64:128)
                    h = 2 * hp + par
                    off = 64 * par
                    acc = ac.tile([128, QB], F32, tag="ac")
                    for kt in range(nkt):
                        scp = sc.tile([128, QB], F32, tag="sc")
                        nc.tensor.matmul(
                            scp[:, :],
                            lhsT=ksb[off:off + 64, kt * 128:(kt + 1) * 128],
                            rhs=qsb[off:off + 64,
                                    hp * 1024 + qh * QB:hp * 1024 + (qh + 1) * QB],
                            start=True, stop=True,
                        )
                        es = ex.tile([128, QB], BF16, tag="ex")
                        nc.scalar.activation(es[:, :], scp[:, :], AF.Exp, scale=SCALE)
                        var = kt - 4 * qh
                        if var >= 0:  # diagonal tile: apply causal mask
                            nc.vector.tensor_mul(
                                es[:, :], es[:, :],
                                masksb[:, var * QB:(var + 1) * QB])
                        nc.tensor.matmul(
                            acc[0:65, :],
                            lhsT=vsb[:, kt * 65:(kt + 1) * 65],
                            rhs=es[:, :],
                            start=(kt == 0), stop=(kt == nkt - 1),
                        )
                    accs.append((h, acc))
                for h, acc in accs:
                    r = rp.tile([1, QB], BF16, tag="rp")
                    with nc.allow_low_precision(reason="softmax 1/sum in bf16"):
                        nc.vector.reciprocal(r[:, :], acc[64:65, :])
                    rbp = sc.tile([128, QB], F32, tag="sc")
                    nc.tensor.matmul(rbp[0:64, :], lhsT=onesb[0:1, 0:64],
                                     rhs=r[0:1, :], start=True, stop=True)
                    rbs = rb.tile([64, QB], BF16, tag="rb")
                    nc.scalar.copy(rbs[:, :], rbp[0:64, :])
                    asb = at.tile([64, QB], BF16, tag="at")
                    nc.vector.tensor_mul(asb[:, :], acc[0:64, :], rbs[:, :])
                    nc.sync.dma_start(ag_in[qh][64 * h:64 * (h + 1), :], asb[:, :])

            nc.gpsimd.collective_compute(
                "AllGather",
                mybir.AluOpType.bypass,
                ins=[ag_in[qh].opt()],
                outs=[ag_out[qh].opt()],
                replica_groups=REPLICA_GROUPS,
            )

        # ---- dense: out^T[e_shard, q] = W_d^T[dpad, e].T @ attn^T[dpad, q] ----
        for qh in range(2):
            gath = gp.tile([128, NET * QB], BF16, tag="gath")
            for i in range(NET):
                nc.sync.dma_start(gath[:, i * QB:(i + 1) * QB],
                                  ag_out[qh][i * 128:(i + 1) * 128, :])
            for m in range(9):
                esz = min(128, ESH - m * 128)
                dps = dp.tile([128, QB], F32, tag="dp")
                for i in range(NET):
                    nc.tensor.matmul(
                        dps[0:esz, :],
                        lhsT=wdsb[:, i * ESH + m * 128:i * ESH + m * 128 + esz],
                        rhs=gath[:, i * QB:(i + 1) * QB],
                        start=(i == 0), stop=(i == NET - 1),
                    )
                osb = op.tile([128, QB], F32, tag="op")
                nc.scalar.copy(osb[0:esz, :], dps[0:esz, :])
                nc.sync.dma_start(out[m * 128:m * 128 + esz, qh * QB:(qh + 1) * QB],
                                  osb[0:esz, :])

    pers.release()
    dram.release()


def build():
    if "nc" in _CACHE:
        return _CACHE["nc"]
    nc = bacc.Bacc("TRN2", target_bir_lowering=False, debug=False,
                   num_devices=NCORES)
    io = {
        "xT": nc.dram_tensor("xT", [D, L], BF16, kind="ExternalInput").ap(),
        "wqkvT": nc.dram_tensor("wqkvT", [D, DLOC + 128], BF16,
                                kind="ExternalInput").ap(),
        "wdT": nc.dram_tensor("wdT", [DPAD, ESH], BF16, kind="ExternalInput").ap(),
        "cosT": nc.dram_tensor("cosT", [128, L], BF16, kind="ExternalInput").ap(),
        "sinT": nc.dram_tensor("sinT", [128, L], BF16, kind="ExternalInput").ap(),
        "rot": nc.dram_tensor("rot", [128, 128], BF16, kind="ExternalInput").ap(),
        "eye": nc.dram_tensor("eye", [128, 128], BF16, kind="ExternalInput").ap(),
        "masks": nc.dram_tensor("masks", [128, 4 * QB], BF16,
                                kind="ExternalInput").ap(),
        "ones64": nc.dram_tensor("ones64", [1, 64], BF16, kind="ExternalInput").ap(),
        "out": nc.dram_tensor("out", [ESH, L], F32, kind="ExternalOutput").ap(),
    }
    with tile.TileContext(nc) as tc:
        _emit(tc, nc, io)
    nc.compile()
    _CACHE["nc"] = nc
    return nc


def make_in_maps(hidden_states, W_qkv, W_dense):
    bf = ml_dtypes.bfloat16
    x = np.asarray(hidden_states, np.float32)
    Wqkv = np.asarray(W_qkv, np.float32)
    Wd = np.asarray(W_dense, np.float32)

    # rope tables, transposed [64, L], replicated to both 64-row halves
    inv = 1.0 / (10000.0 ** (np.arange(0, DKV, 2, dtype=np.float32) / DKV))
    t = np.arange(L, dtype=np.float32)
    freqs = np.outer(t, inv)
    emb = np.concatenate([freqs, freqs], axis=1)          # [L, 64]
    cosT = np.tile(np.cos(emb).T, (2, 1)).astype(bf)      # [128, L]
    sinT = np.tile(np.sin(emb).T, (2, 1)).astype(bf)

    # rotate_half as a matmul: qrot = R1 @ q; lhsT = R1^T; 2-head block diagonal
    R1 = np.zeros((DKV, DKV), np.float32)
    for i in range(32):
        R1[i, i + 32] = -1.0
        R1[i + 32, i] = 1.0
    R2 = np.zeros((128, 128), np.float32)
    R2[:64, :64] = R1
    R2[64:, 64:] = R1
    rot = R2.T.copy().astype(bf)

    eye = np.eye(128, dtype=np.float32).astype(bf)
    ones64 = np.ones((1, 64), np.float32).astype(bf)

    # causal masks for diagonal tiles: variant j (k-tile j within the q-block)
    masks = np.zeros((128, 4, QB), np.float32)
    kk = np.arange(128)[:, None]
    qq = np.arange(QB)[None, :]
    for j in range(4):
        masks[:, j, :] = (128 * j + kk) <= qq
    masks = masks.reshape(128, 4 * QB).astype(bf)

    # padded dense weights: W_d^T with 64 zero rows appended (pad head)
    wdT_full = np.concatenate([Wd.T, np.zeros((DPAD - D, D), np.float32)], axis=0)
    wdT_full = wdT_full.astype(bf)

    WkvT = Wqkv[H * DKV:].T.astype(bf)                    # [D, 128]

    in_maps = []
    for c in range(NCORES):
        g, j = divmod(c, GSZ)
        h0 = HPC * j
        nh = HPC if j < GSZ - 1 else H - HPC * (GSZ - 1)  # 18,18,18,17
        WqT = np.zeros((D, DLOC), np.float32)
        WqT[:, :nh * DKV] = Wqkv[DKV * h0:DKV * (h0 + nh)].T
        in_maps.append({
            "xT": np.ascontiguousarray(x[g].T).astype(bf),
            "wqkvT": np.concatenate([WqT.astype(bf), WkvT], axis=1),
            "wdT": np.ascontiguousarray(wdT_full[:, ESH * j:ESH * (j + 1)]),
            "cosT": cosT, "sinT": sinT, "rot": rot, "eye": eye,
            "masks": masks, "ones64": ones64,
        })
    return in_maps


def assemble(results):
    out = np.empty((N, L, D), np.float32)
    for c in range(NCORES):
        g, j = divmod(c, GSZ)
        out[g, :, ESH * j:ESH * (j + 1)] = results[c]["out"].T
    return out


def kernel(hidden_states, W_qkv, W_dense):
    nc = build()
    in_maps = make_in_maps(hidden_states, W_qkv, W_dense)
    res = run_bass_kernel_spmd(nc, in_maps, core_ids=list(range(NCORES)))
    return assemble(res.results)


if __name__ == "__main__":
    import reference
    inputs = reference.setup_inputs()
    out = kernel(**{k: np.asarray(v) for k, v in inputs.items()})
    print("out", out.shape, out.dtype)


# revision 17
# speedup vs baseline: 1.0442x; 1.0442x over previous
"""Distributed MQA attention block (N=2, L=1024, D=4544, H=71, Dkv=64) on 8 TRN2 cores.

Sharding: 2 batch groups x 4-way head tensor-parallel.
  core c = 4*g + j: batch g, heads [18j, 18j+18) (core j=3: 17 real heads + 1 zero pad).
Per core: QKV projection (bf16), RoPE (rotation via PE matmul), causal attention in
S^T = K@Q^T orientation (softmax sum fused into the V-matmul via an appended
ones-column), AllGather of attn^T (bf16) within each 4-core group per q-half,
column-sharded dense projection. Host casts inputs to bf16 / pre-transposes, and
assembles the 8 [1136, 1024] f32 output shards.
"""

import sys

if "/opt/trn_rl_repo" not in sys.path:
    sys.path.insert(0, "/opt/trn_rl_repo")

import numpy as np
import ml_dtypes

import concourse.bass as bass
import concourse.bacc as bacc
import concourse.mybir as mybir
import concourse.tile as tile
from concourse.bass_utils import run_bass_kernel_spmd

BF16 = mybir.dt.bfloat16
F32 = mybir.dt.float32
AF = mybir.ActivationFunctionType

N, L, D = 2, 1024, 4544
H, DKV = 71, 64
NCORES, GSZ = 8, 4
HPC = 18                 # heads per core (last core of each group: 17 real + 1 pad)
DLOC = HPC * DKV         # 1152
DPAD = GSZ * DLOC        # 4608 = 36 * 128
ESH = D // GSZ           # 1136 output-column shard
NET = 36                 # e-contraction tiles over D=4544 (35 x 128 + 1 x 64)
NMT = DLOC // 128        # 9 m-tiles of Q^T rows (2 heads each)
QB = 512                 # q-block (half of L)
SCALE = 1.0 / np.sqrt(DKV)
REPLICA_GROUPS = [[0, 1, 2, 3], [4, 5, 6, 7]]

_CACHE = {}


def _esz(i):
    return 128 if i < NET - 1 else 64


def _emit(tc, nc, io):
    xT, wqkvT, wdT, cosT, sinT, rot, eye, masks, ones64, out = (
        io["xT"], io["wqkvT"], io["wdT"], io["cosT"], io["sinT"], io["rot"],
        io["eye"], io["masks"], io["ones64"], io["out"],
    )

    # ---- persistent SBUF (live through the whole kernel) ----
    pers = tc.alloc_tile_pool(name="pers", bufs=1)
    qsb = pers.tile([128, NMT * 1024], BF16, tag="qsb")    # roped Q^T, 2 heads/tile
    ksb = pers.tile([128, 1024], BF16, tag="ksb")          # roped K^T, dup in both halves
    vsb = pers.tile([128, 8 * 65], BF16, tag="vsb")        # V [tok,64]+ones col, 8 chunks
    cossb = pers.tile([128, 1024], BF16, tag="cossb")
    sinsb = pers.tile([128, 1024], BF16, tag="sinsb")
    rotsb = pers.tile([128, 128], BF16, tag="rotsb")
    eyesb = pers.tile([128, 128], BF16, tag="eyesb")
    masksb = pers.tile([128, 128], BF16, tag="masksb")
    onesb = pers.tile([1, 64], BF16, tag="onesb")

    nc.sync.dma_start(cossb[:, :], cosT[:, :])
    nc.sync.dma_start(sinsb[:, :], sinT[:, :])
    nc.sync.dma_start(rotsb[:, :], rot[:, :])
    nc.sync.dma_start(eyesb[:, :], eye[:, :])
    nc.sync.dma_start(masksb[:, :], masks[:, :])
    nc.sync.dma_start(onesb[:, :], ones64[:, :])
    nc.vector.memset(vsb[:, :], 1.0)  # ones column survives the V copies below

    # ---- DRAM bounce buffers for the per-q-half AllGather ----
    dram = tc.alloc_tile_pool(name="dram", bufs=1, space="DRAM")
    ag_in = [dram.tile([DLOC, QB], BF16, tag=f"agin{q}", name=f"agin{q}")
             for q in range(2)]
    ag_out = [dram.tile([DPAD, QB], BF16, tag=f"agout{q}", name=f"agout{q}")
              for q in range(2)]

    # ================= Phase A: QKV projection + RoPE =================
    with (
        tc.tile_pool(name="xp", bufs=1) as xp,
        tc.tile_pool(name="wqp", bufs=2) as wqp,
        tc.tile_pool(name="ra", bufs=2) as ra,
        tc.tile_pool(name="rt", bufs=2) as rt,
        tc.tile_pool(name="vt", bufs=1) as vt,
        tc.tile_pool(name="ps2", bufs=3, space="PSUM") as ps2,
    ):
        xsb = xp.tile([128, NET * 1024], BF16, tag="xsb")
        for i in range(NET):
            p = _esz(i)
            nc.sync.dma_start(xsb[0:p, i * 1024:(i + 1) * 1024],
                              xT[i * 128:i * 128 + p, :])

        def load_w_cols(dst, c0, cw):
            # wqkvT[:, c0:c0+cw] -> dst [128, NET*cw] (tile i at cols i*cw)
            nc.sync.dma_start(
                dst[:, 0:(NET - 1) * cw].rearrange("p (t c) -> p t c", c=cw),
                wqkvT[0:(NET - 1) * 128, c0:c0 + cw].rearrange(
                    "(t p) c -> p t c", p=128),
            )
            nc.sync.dma_start(dst[0:64, (NET - 1) * cw:NET * cw],
                              wqkvT[(NET - 1) * 128:D, c0:c0 + cw])

        # ---- K/V projections (shared KV head) ----
        wkv = wqp.tile([128, NET * 128], BF16, tag="wm")
        load_w_cols(wkv, DLOC, 128)

        # K^T [64, 1024] (psum, 2 banks)
        kps = ps2.tile([128, 1024], F32, tag="ps2")
        for i in range(NET):
            p = _esz(i)
            for q in range(2):
                nc.tensor.matmul(
                    kps[0:64, q * QB:(q + 1) * QB],
                    lhsT=wkv[0:p, i * 128:i * 128 + 64],
                    rhs=xsb[0:p, i * 1024 + q * QB:i * 1024 + (q + 1) * QB],
                    start=(i == 0), stop=(i == NET - 1),
                )
        # rope K (rows 0:64), then duplicate into rows 64:128 via DMA
        kraw = ra.tile([128, 1024], BF16, tag="ra")
        nc.scalar.copy(kraw[0:64, :], kps[0:64, :])
        krot = ps2.tile([128, 1024], F32, tag="ps2")
        for q in range(2):
            nc.tensor.matmul(krot[0:64, q * QB:(q + 1) * QB],
                             lhsT=rotsb[0:64, 0:64],
                             rhs=kraw[0:64, q * QB:(q + 1) * QB],
                             start=True, stop=True)
        for q in range(2):
            s = slice(q * QB, (q + 1) * QB)
            t1 = rt.tile([128, QB], F32, tag="t1")
            t2 = rt.tile([128, QB], F32, tag="t2")
            nc.vector.tensor_mul(t1[0:64, :], kraw[0:64, s], cossb[0:64, s])
            nc.vector.tensor_mul(t2[0:64, :], krot[0:64, s], sinsb[0:64, s])
            nc.vector.tensor_add(ksb[0:64, s], t1[0:64, :], t2[0:64, :])
        nc.sync.dma_start(ksb[64:128, :], ksb[0:64, :])

        # V^T [64, 1024] then transpose to V [tok, 64] chunks in vsb
        vps = ps2.tile([128, 1024], F32, tag="ps2")
        for i in range(NET):
            p = _esz(i)
            for q in range(2):
                nc.tensor.matmul(
                    vps[0:64, q * QB:(q + 1) * QB],
                    lhsT=wkv[0:p, i * 128 + 64:i * 128 + 128],
                    rhs=xsb[0:p, i * 1024 + q * QB:i * 1024 + (q + 1) * QB],
                    start=(i == 0), stop=(i == NET - 1),
                )
        vtsb = vt.tile([64, 1024], BF16, tag="vtsb")
        nc.scalar.copy(vtsb[:, :], vps[0:64, :])
        for t8 in range(8):
            vtp = ps2.tile([128, 2048], BF16, tag="ps2")
            nc.tensor.transpose(vtp[0:128, 0:64],
                                vtsb[0:64, t8 * 128:(t8 + 1) * 128],
                                eyesb[0:64, 0:64])
            nc.scalar.copy(vsb[:, t8 * 65:t8 * 65 + 64], vtp[0:128, 0:64])

        # ---- Q projection + RoPE, per m-tile (2 heads each) ----
        for m in range(NMT):
            wqm = wqp.tile([128, NET * 128], BF16, tag="wm")
            load_w_cols(wqm, m * 128, 128)
            qps = ps2.tile([128, 1024], F32, tag="ps2")
            for i in range(NET):
                p = _esz(i)
                for q in range(2):
                    nc.tensor.matmul(
                        qps[:, q * QB:(q + 1) * QB],
                        lhsT=wqm[0:p, i * 128:(i + 1) * 128],
                        rhs=xsb[0:p, i * 1024 + q * QB:i * 1024 + (q + 1) * QB],
                        start=(i == 0), stop=(i == NET - 1),
                    )
            qraw = ra.tile([128, 1024], BF16, tag="ra")
            nc.scalar.copy(qraw[:, :], qps[:, :])
            qrot = ps2.tile([128, 1024], F32, tag="ps2")
            for q in range(2):
                nc.tensor.matmul(qrot[:, q * QB:(q + 1) * QB],
                                 lhsT=rotsb[:, :],
                                 rhs=qraw[:, q * QB:(q + 1) * QB],
                                 start=True, stop=True)
            for q in range(2):
                s = slice(q * QB, (q + 1) * QB)
                t1 = rt.tile([128, QB], F32, tag="t1")
                t2 = rt.tile([128, QB], F32, tag="t2")
                nc.vector.tensor_mul(t1[:, :], qraw[:, s], cossb[:, s])
                nc.vector.tensor_mul(t2[:, :], qrot[:, s], sinsb[:, s])
                nc.vector.tensor_add(qsb[:, m * 1024 + q * QB:m * 1024 + (q + 1) * QB],
                                     t1[:, :], t2[:, :])

    # ================= Phase B: attention + AllGather + dense =================
    with (
        tc.tile_pool(name="ex", bufs=18) as ex,
        tc.tile_pool(name="at", bufs=4) as at,
        tc.tile_pool(name="rb", bufs=2) as rb,
        tc.tile_pool(name="rp", bufs=2) as rp,
        tc.tile_pool(name="rpf", bufs=2) as rpf,
        tc.tile_pool(name="wdp", bufs=1) as wdp,
        tc.tile_pool(name="gp", bufs=1) as gp,
        tc.tile_pool(name="op", bufs=3) as op,
        tc.tile_pool(name="sc", bufs=3, space="PSUM") as sc,
        tc.tile_pool(name="ac", bufs=4, space="PSUM") as ac,
        tc.tile_pool(name="dp", bufs=1, space="PSUM") as dp,
    ):
        # dense weights resident; DMAs run in the background during attention
        wdsb = wdp.tile([128, NET * ESH], BF16, tag="wdsb")
        for i in range(NET):
            nc.sync.dma_start(wdsb[:, i * ESH:(i + 1) * ESH],
                              wdT[i * 128:(i + 1) * 128, :])

        for qh in range(2):
            nkt = 4 * qh + 4
            for hp in range(NMT):
                # --- S burst: scores + exp for both heads of the pair; the
                # diagonal k-tile of q-block qh is k-tile 4*qh+j: columns
                # below 128*j are fully masked (skip compute, memset 0),
                # the [128j, 128j+128) block gets the triangular mask.
                ess = [[None] * nkt for _ in range(2)]
                for par in range(2):
                    off = 64 * par
                    for kt in range(nkt):
                        var = kt - 4 * qh
                        c0 = 128 * var if var > 0 else 0
                        scp = sc.tile([128, QB], F32, tag="sc")
                        nc.tensor.matmul(
                            scp[:, c0:QB],
                            lhsT=ksb[off:off + 64, kt * 128:(kt + 1) * 128],
                            rhs=qsb[off:off + 64,
                                    hp * 1024 + qh * QB + c0:hp * 1024 + (qh + 1) * QB],
                            start=True, stop=True,
                        )
                        es = ex.tile([128, QB], BF16, tag="ex")
                        if c0 > 0:
                            nc.vector.memset(es[:, 0:c0], 0.0)
                        nc.scalar.activation(es[:, c0:QB], scp[:, c0:QB],
                                             AF.Exp, scale=SCALE)
                        if var >= 0:  # triangular mask on the diagonal block
                            nc.vector.tensor_mul(
                                es[:, c0:c0 + 128], es[:, c0:c0 + 128],
                                masksb[:, 0:128])
                        ess[par][kt] = es
                # --- V burst: back-to-back accumulating matmuls on PE
                accs = []
                for par in range(2):
                    h = 2 * hp + par
                    acc = ac.tile([128, QB], F32, tag="ac")
                    for kt in range(nkt):
                        nc.tensor.matmul(
                            acc[0:65, :],
                            lhsT=vsb[:, kt * 65:(kt + 1) * 65],
                            rhs=ess[par][kt][:, :],
                            start=(kt == 0), stop=(kt == nkt - 1),
                        )
                    accs.append((h, acc))
                # --- normalize: 1/sum (row 64), broadcast via K=1 matmul
                for h, acc in accs:
                    r = rp.tile([1, QB], BF16, tag="rp")
                    with nc.allow_low_precision(reason="softmax 1/sum in bf16"):
                        nc.vector.reciprocal(r[:, :], acc[64:65, :])
                    rbp = sc.tile([128, QB], F32, tag="sc")
                    nc.tensor.matmul(rbp[0:64, :], lhsT=onesb[0:1, 0:64],
                                     rhs=r[0:1, :], start=True, stop=True)
                    rbs = rb.tile([64, QB], BF16, tag="rb")
                    nc.scalar.copy(rbs[:, :], rbp[0:64, :])
                    asb = at.tile([64, QB], BF16, tag="at")
                    nc.vector.tensor_mul(asb[:, :], acc[0:64, :], rbs[:, :])
                    nc.sync.dma_start(ag_in[qh][64 * h:64 * (h + 1), :], asb[:, :])

            nc.gpsimd.collective_compute(
                "AllGather",
                mybir.AluOpType.bypass,
                ins=[ag_in[qh].opt()],
                outs=[ag_out[qh].opt()],
                replica_groups=REPLICA_GROUPS,
            )

        # ---- dense: out^T[e_shard, q] = W_d^T[dpad, e].T @ attn^T[dpad, q] ----
        for qh in range(2):
            gath = gp.tile([128, NET * QB], BF16, tag="gath")
            for i in range(NET):
                nc.sync.dma_start(gath[:, i * QB:(i + 1) * QB],
                                  ag_out[qh][i * 128:(i + 1) * 128, :])
            for m in range(9):
                esz = min(128, ESH - m * 128)
                dps = dp.tile([128, QB], F32, tag="dp")
                for i in range(NET):
                    nc.tensor.matmul(
                        dps[0:esz, :],
                        lhsT=wdsb[:, i * ESH + m * 128:i * ESH + m * 128 + esz],
                        rhs=gath[:, i * QB:(i + 1) * QB],
                        start=(i == 0), stop=(i == NET - 1),
                    )
                osb = op.tile([128, QB], F32, tag="op")
                nc.scalar.copy(osb[0:esz, :], dps[0:esz, :])
                nc.sync.dma_start(out[m * 128:m * 128 + esz, qh * QB:(qh + 1) * QB],
                                  osb[0:esz, :])

    pers.release()
    dram.release()


def build():
    if "nc" in _CACHE:
        return _CACHE["nc"]
    nc = bacc.Bacc("TRN2", target_bir_lowering=False, debug=False,
                   num_devices=NCORES)
    io = {
        "xT": nc.dram_tensor("xT", [D, L], BF16, kind="ExternalInput").ap(),
        "wqkvT": nc.dram_tensor("wqkvT", [D, DLOC + 128], BF16,
                                kind="ExternalInput").ap(),
        "wdT": nc.dram_tensor("wdT", [DPAD, ESH], BF16, kind="ExternalInput").ap(),
        "cosT": nc.dram_tensor("cosT", [128, L], BF16, kind="ExternalInput").ap(),
        "sinT": nc.dram_tensor("sinT", [128, L], BF16, kind="ExternalInput").ap(),
        "rot": nc.dram_tensor("rot", [128, 128], BF16, kind="ExternalInput").ap(),
        "eye": nc.dram_tensor("eye", [128, 128], BF16, kind="ExternalInput").ap(),
        "masks": nc.dram_tensor("masks", [128, 128], BF16,
                                kind="ExternalInput").ap(),
        "ones64": nc.dram_tensor("ones64", [1, 64], BF16, kind="ExternalInput").ap(),
        "out": nc.dram_tensor("out", [ESH, L], F32, kind="ExternalOutput").ap(),
    }
    with tile.TileContext(nc) as tc:
        _emit(tc, nc, io)
    nc.compile()
    _CACHE["nc"] = nc
    return nc


def make_in_maps(hidden_states, W_qkv, W_dense):
    bf = ml_dtypes.bfloat16
    x = np.asarray(hidden_states, np.float32)
    Wqkv = np.asarray(W_qkv, np.float32)
    Wd = np.asarray(W_dense, np.float32)

    # rope tables, transposed [64, L], replicated to both 64-row halves
    inv = 1.0 / (10000.0 ** (np.arange(0, DKV, 2, dtype=np.float32) / DKV))
    t = np.arange(L, dtype=np.float32)
    freqs = np.outer(t, inv)
    emb = np.concatenate([freqs, freqs], axis=1)          # [L, 64]
    cosT = np.tile(np.cos(emb).T, (2, 1)).astype(bf)      # [128, L]
    sinT = np.tile(np.sin(emb).T, (2, 1)).astype(bf)

    # rotate_half as a matmul: qrot = R1 @ q; lhsT = R1^T; 2-head block diagonal
    R1 = np.zeros((DKV, DKV), np.float32)
    for i in range(32):
        R1[i, i + 32] = -1.0
        R1[i + 32, i] = 1.0
    R2 = np.zeros((128, 128), np.float32)
    R2[:64, :64] = R1
    R2[64:, 64:] = R1
    rot = R2.T.copy().astype(bf)

    eye = np.eye(128, dtype=np.float32).astype(bf)
    ones64 = np.ones((1, 64), np.float32).astype(bf)

    # triangular causal mask for the 128x128 diagonal block
    kk = np.arange(128)[:, None]
    qq = np.arange(128)[None, :]
    masks = (kk <= qq).astype(np.float32).astype(bf)

    # padded dense weights: W_d^T with 64 zero rows appended (pad head)
    wdT_full = np.concatenate([Wd.T, np.zeros((DPAD - D, D), np.float32)], axis=0)
    wdT_full = wdT_full.astype(bf)

    WkvT = Wqkv[H * DKV:].T.astype(bf)                    # [D, 128]

    in_maps = []
    for c in range(NCORES):
        g, j = divmod(c, GSZ)
        h0 = HPC * j
        nh = HPC if j < GSZ - 1 else H - HPC * (GSZ - 1)  # 18,18,18,17
        WqT = np.zeros((D, DLOC), np.float32)
        WqT[:, :nh * DKV] = Wqkv[DKV * h0:DKV * (h0 + nh)].T
        in_maps.append({
            "xT": np.ascontiguousarray(x[g].T).astype(bf),
            "wqkvT": np.concatenate([WqT.astype(bf), WkvT], axis=1),
            "wdT": np.ascontiguousarray(wdT_full[:, ESH * j:ESH * (j + 1)]),
            "cosT": cosT, "sinT": sinT, "rot": rot, "eye": eye,
            "masks": masks, "ones64": ones64,
        })
    return in_maps


def assemble(results):
    out = np.empty((N, L, D), np.float32)
    for c in range(NCORES):
        g, j = divmod(c, GSZ)
        out[g, :, ESH * j:ESH * (j + 1)] = results[c]["out"].T
    return out


def kernel(hidden_states, W_qkv, W_dense):
    nc = build()
    in_maps = make_in_maps(hidden_states, W_qkv, W_dense)
    res = run_bass_kernel_spmd(nc, in_maps, core_ids=list(range(NCORES)))
    return assemble(res.results)


if __name__ == "__main__":
    import reference
    inputs = reference.setup_inputs()
    out = kernel(**{k: np.asarray(v) for k, v in inputs.items()})
    print("out", out.shape, out.dtype)


# revision 19
# speedup vs baseline: 1.1237x; 1.0761x over previous
"""Distributed MQA attention block (N=2, L=1024, D=4544, H=71, Dkv=64) on 8 TRN2 cores.

Sharding: 2 batch groups x 4-way head tensor-parallel.
  core c = 4*g + j: batch g, heads [18j, 18j+18) (core j=3: 17 real heads + 1 zero pad).
Per core: QKV projection (bf16), RoPE (rotation via PE matmul), causal attention in
S^T = K@Q^T orientation (softmax sum fused into the V-matmul via an appended
ones-column), AllGather of attn^T (bf16) within each 4-core group per q-half,
column-sharded dense projection. Host casts inputs to bf16 / pre-transposes, and
assembles the 8 [1136, 1024] f32 output shards.
"""

import sys

if "/opt/trn_rl_repo" not in sys.path:
    sys.path.insert(0, "/opt/trn_rl_repo")

import numpy as np
import ml_dtypes

import concourse.bass as bass
import concourse.bacc as bacc
import concourse.mybir as mybir
import concourse.tile as tile
from concourse.bass_utils import run_bass_kernel_spmd

BF16 = mybir.dt.bfloat16
F32 = mybir.dt.float32
AF = mybir.ActivationFunctionType

N, L, D = 2, 1024, 4544
H, DKV = 71, 64
NCORES, GSZ = 8, 4
HPC = 18                 # heads per core (last core of each group: 17 real + 1 pad)
DLOC = HPC * DKV         # 1152
DPAD = GSZ * DLOC        # 4608 = 36 * 128
ESH = D // GSZ           # 1136 output-column shard
NET = 36                 # e-contraction tiles over D=4544 (35 x 128 + 1 x 64)
NMT = DLOC // 128        # 9 m-tiles of Q^T rows (2 heads each)
QB = 512                 # q-block (half of L)
SCALE = 1.0 / np.sqrt(DKV)
REPLICA_GROUPS = [[0, 1, 2, 3], [4, 5, 6, 7]]

_CACHE = {}


def _esz(i):
    return 128 if i < NET - 1 else 64


def _emit(tc, nc, io):
    xT, wqkvT, wdT, cosT, sinT, rot, eye, masks, ones64, out = (
        io["xT"], io["wqkvT"], io["wdT"], io["cosT"], io["sinT"], io["rot"],
        io["eye"], io["masks"], io["ones64"], io["out"],
    )

    # ---- persistent SBUF (live through the whole kernel) ----
    pers = tc.alloc_tile_pool(name="pers", bufs=1)
    qsb = pers.tile([128, NMT * 1024], BF16, tag="qsb")    # roped Q^T, 2 heads/tile
    ksb = pers.tile([128, 1024], BF16, tag="ksb")          # roped K^T, dup in both halves
    vsb = pers.tile([128, 8 * 65], BF16, tag="vsb")        # V [tok,64]+ones col, 8 chunks
    cossb = pers.tile([128, 1024], BF16, tag="cossb")
    sinsb = pers.tile([128, 1024], BF16, tag="sinsb")
    rotsb = pers.tile([128, 128], BF16, tag="rotsb")
    eyesb = pers.tile([128, 128], BF16, tag="eyesb")
    masksb = pers.tile([128, 128], BF16, tag="masksb")
    onesb = pers.tile([1, 64], BF16, tag="onesb")

    nc.sync.dma_start(cossb[:, :], cosT[:, :])
    nc.sync.dma_start(sinsb[:, :], sinT[:, :])
    nc.sync.dma_start(rotsb[:, :], rot[:, :])
    nc.sync.dma_start(eyesb[:, :], eye[:, :])
    nc.sync.dma_start(masksb[:, :], masks[:, :])
    nc.sync.dma_start(onesb[:, :], ones64[:, :])
    nc.vector.memset(vsb[:, :], 1.0)  # ones column survives the V copies below

    # ---- DRAM bounce buffers for the per-q-half AllGather ----
    dram = tc.alloc_tile_pool(name="dram", bufs=1, space="DRAM")
    ag_in = [dram.tile([DLOC, QB], BF16, tag=f"agin{q}", name=f"agin{q}")
             for q in range(2)]
    ag_out = [dram.tile([DPAD, QB], BF16, tag=f"agout{q}", name=f"agout{q}")
              for q in range(2)]

    # ================= Phase A: QKV projection + RoPE =================
    with (
        tc.tile_pool(name="xp", bufs=1) as xp,
        tc.tile_pool(name="wqp", bufs=2) as wqp,
        tc.tile_pool(name="ra", bufs=2) as ra,
        tc.tile_pool(name="rt", bufs=2) as rt,
        tc.tile_pool(name="vt", bufs=1) as vt,
        tc.tile_pool(name="ps2", bufs=3, space="PSUM") as ps2,
    ):
        xsb = xp.tile([128, NET * 1024], BF16, tag="xsb")
        for i in range(NET):
            p = _esz(i)
            nc.sync.dma_start(xsb[0:p, i * 1024:(i + 1) * 1024],
                              xT[i * 128:i * 128 + p, :])

        def load_w_cols(dst, c0, cw):
            # wqkvT[:, c0:c0+cw] -> dst [128, NET*cw] (tile i at cols i*cw)
            nc.sync.dma_start(
                dst[:, 0:(NET - 1) * cw].rearrange("p (t c) -> p t c", c=cw),
                wqkvT[0:(NET - 1) * 128, c0:c0 + cw].rearrange(
                    "(t p) c -> p t c", p=128),
            )
            nc.sync.dma_start(dst[0:64, (NET - 1) * cw:NET * cw],
                              wqkvT[(NET - 1) * 128:D, c0:c0 + cw])

        # ---- K/V projections (shared KV head) ----
        wkv = wqp.tile([128, NET * 128], BF16, tag="wm")
        load_w_cols(wkv, DLOC, 128)

        # K^T [64, 1024] (psum, 2 banks)
        kps = ps2.tile([128, 1024], F32, tag="ps2")
        for i in range(NET):
            p = _esz(i)
            for q in range(2):
                nc.tensor.matmul(
                    kps[0:64, q * QB:(q + 1) * QB],
                    lhsT=wkv[0:p, i * 128:i * 128 + 64],
                    rhs=xsb[0:p, i * 1024 + q * QB:i * 1024 + (q + 1) * QB],
                    start=(i == 0), stop=(i == NET - 1),
                )
        # rope K (rows 0:64), then duplicate into rows 64:128 via DMA
        kraw = ra.tile([128, 1024], BF16, tag="ra")
        nc.scalar.copy(kraw[0:64, :], kps[0:64, :])
        krot = ps2.tile([128, 1024], F32, tag="ps2")
        for q in range(2):
            nc.tensor.matmul(krot[0:64, q * QB:(q + 1) * QB],
                             lhsT=rotsb[0:64, 0:64],
                             rhs=kraw[0:64, q * QB:(q + 1) * QB],
                             start=True, stop=True)
        for q in range(2):
            s = slice(q * QB, (q + 1) * QB)
            t1 = rt.tile([128, QB], F32, tag="t1")
            t2 = rt.tile([128, QB], F32, tag="t2")
            nc.vector.tensor_mul(t1[0:64, :], kraw[0:64, s], cossb[0:64, s])
            nc.vector.tensor_mul(t2[0:64, :], krot[0:64, s], sinsb[0:64, s])
            nc.vector.tensor_add(ksb[0:64, s], t1[0:64, :], t2[0:64, :])
        nc.sync.dma_start(ksb[64:128, :], ksb[0:64, :])

        # V^T [64, 1024] then transpose to V [tok, 64] chunks in vsb
        vps = ps2.tile([128, 1024], F32, tag="ps2")
        for i in range(NET):
            p = _esz(i)
            for q in range(2):
                nc.tensor.matmul(
                    vps[0:64, q * QB:(q + 1) * QB],
                    lhsT=wkv[0:p, i * 128 + 64:i * 128 + 128],
                    rhs=xsb[0:p, i * 1024 + q * QB:i * 1024 + (q + 1) * QB],
                    start=(i == 0), stop=(i == NET - 1),
                )
        vtsb = vt.tile([64, 1024], BF16, tag="vtsb")
        nc.scalar.copy(vtsb[:, :], vps[0:64, :])
        for t8 in range(8):
            vtp = ps2.tile([128, 2048], BF16, tag="ps2")
            nc.tensor.transpose(vtp[0:128, 0:64],
                                vtsb[0:64, t8 * 128:(t8 + 1) * 128],
                                eyesb[0:64, 0:64])
            nc.scalar.copy(vsb[:, t8 * 65:t8 * 65 + 64], vtp[0:128, 0:64])

        # ---- Q projection + RoPE, per m-tile (2 heads each) ----
        for m in range(NMT):
            wqm = wqp.tile([128, NET * 128], BF16, tag="wm")
            load_w_cols(wqm, m * 128, 128)
            qps = ps2.tile([128, 1024], F32, tag="ps2")
            for i in range(NET):
                p = _esz(i)
                for q in range(2):
                    nc.tensor.matmul(
                        qps[:, q * QB:(q + 1) * QB],
                        lhsT=wqm[0:p, i * 128:(i + 1) * 128],
                        rhs=xsb[0:p, i * 1024 + q * QB:i * 1024 + (q + 1) * QB],
                        start=(i == 0), stop=(i == NET - 1),
                    )
            qraw = ra.tile([128, 1024], BF16, tag="ra")
            nc.scalar.copy(qraw[:, :], qps[:, :])
            qrot = ps2.tile([128, 1024], F32, tag="ps2")
            for q in range(2):
                nc.tensor.matmul(qrot[:, q * QB:(q + 1) * QB],
                                 lhsT=rotsb[:, :],
                                 rhs=qraw[:, q * QB:(q + 1) * QB],
                                 start=True, stop=True)
            for q in range(2):
                s = slice(q * QB, (q + 1) * QB)
                t1 = rt.tile([128, QB], F32, tag="t1")
                t2 = rt.tile([128, QB], F32, tag="t2")
                nc.vector.tensor_mul(t1[:, :], qraw[:, s], cossb[:, s])
                nc.vector.tensor_mul(t2[:, :], qrot[:, s], sinsb[:, s])
                nc.vector.tensor_add(qsb[:, m * 1024 + q * QB:m * 1024 + (q + 1) * QB],
                                     t1[:, :], t2[:, :])

    # ================= Phase B: attention + AllGather + dense =================
    with (
        tc.tile_pool(name="ex", bufs=18) as ex,
        tc.tile_pool(name="at", bufs=4) as at,
        tc.tile_pool(name="rb", bufs=2) as rb,
        tc.tile_pool(name="rp", bufs=2) as rp,
        tc.tile_pool(name="rpf", bufs=2) as rpf,
        tc.tile_pool(name="wdp", bufs=1) as wdp,
        tc.tile_pool(name="gp", bufs=1) as gp,
        tc.tile_pool(name="op", bufs=3) as op,
        tc.tile_pool(name="sc", bufs=3, space="PSUM") as sc,
        tc.tile_pool(name="ac", bufs=4, space="PSUM") as ac,
        tc.tile_pool(name="dp", bufs=1, space="PSUM") as dp,
    ):
        # dense weights resident; DMAs run in the background during attention
        wdsb = wdp.tile([128, NET * ESH], BF16, tag="wdsb")
        for i in range(NET):
            nc.sync.dma_start(wdsb[:, i * ESH:(i + 1) * ESH],
                              wdT[i * 128:(i + 1) * 128, :])

        for qh in range(2):
            nkt = 4 * qh + 4
            for hp in range(NMT):
                # --- S burst: scores + exp for both heads of the pair; the
                # diagonal k-tile of q-block qh is k-tile 4*qh+j: columns
                # below 128*j are fully masked (skip compute, memset 0),
                # the [128j, 128j+128) block gets the triangular mask.
                ess = [[None] * nkt for _ in range(2)]
                for par in range(2):
                    off = 64 * par
                    for kt in range(nkt):
                        var = kt - 4 * qh
                        c0 = 128 * var if var > 0 else 0
                        scp = sc.tile([128, QB], F32, tag="sc")
                        nc.tensor.matmul(
                            scp[:, c0:QB],
                            lhsT=ksb[off:off + 64, kt * 128:(kt + 1) * 128],
                            rhs=qsb[off:off + 64,
                                    hp * 1024 + qh * QB + c0:hp * 1024 + (qh + 1) * QB],
                            start=True, stop=True,
                        )
                        es = ex.tile([128, QB], BF16, tag="ex")
                        if c0 > 0:
                            nc.vector.memset(es[:, 0:c0], 0.0)
                        nc.scalar.activation(es[:, c0:QB], scp[:, c0:QB],
                                             AF.Exp, scale=SCALE)
                        if var >= 0:  # triangular mask on the diagonal block
                            nc.gpsimd.tensor_mul(
                                es[:, c0:c0 + 128], es[:, c0:c0 + 128],
                                masksb[:, 0:128])
                        ess[par][kt] = es
                # --- V burst: back-to-back accumulating matmuls on PE
                accs = []
                for par in range(2):
                    h = 2 * hp + par
                    acc = ac.tile([128, QB], F32, tag="ac")
                    for kt in range(nkt):
                        nc.tensor.matmul(
                            acc[0:65, :],
                            lhsT=vsb[:, kt * 65:(kt + 1) * 65],
                            rhs=ess[par][kt][:, :],
                            start=(kt == 0), stop=(kt == nkt - 1),
                        )
                    accs.append((h, acc))
                # --- normalize: 1/sum (row 64), broadcast via K=1 matmul
                for h, acc in accs:
                    stg = rpf.tile([1, QB], F32, tag="stg")
                    nc.scalar.copy(stg[:, :], acc[64:65, :])
                    rf = rpf.tile([1, QB], F32, tag="rpf")
                    nc.vector.reciprocal_approx_fast(rf[:, :], stg[:, :])
                    r = rp.tile([1, QB], BF16, tag="rp")
                    with nc.allow_low_precision(reason="softmax 1/sum in bf16"):
                        nc.vector.tensor_copy(r[:, :], rf[:, :])
                    rbp = sc.tile([128, QB], F32, tag="sc")
                    nc.tensor.matmul(rbp[0:64, :], lhsT=onesb[0:1, 0:64],
                                     rhs=r[0:1, :], start=True, stop=True)
                    rbs = rb.tile([64, QB], BF16, tag="rb")
                    nc.vector.tensor_copy(rbs[:, :], rbp[0:64, :])
                    asb = at.tile([64, QB], BF16, tag="at")
                    nc.vector.tensor_mul(asb[:, :], acc[0:64, :], rbs[:, :])
                    nc.sync.dma_start(ag_in[qh][64 * h:64 * (h + 1), :], asb[:, :])

            nc.gpsimd.collective_compute(
                "AllGather",
                mybir.AluOpType.bypass,
                ins=[ag_in[qh].opt()],
                outs=[ag_out[qh].opt()],
                replica_groups=REPLICA_GROUPS,
            )

        # ---- dense: out^T[e_shard, q] = W_d^T[dpad, e].T @ attn^T[dpad, q] ----
        for qh in range(2):
            gath = gp.tile([128, NET * QB], BF16, tag="gath")
            for i in range(NET):
                nc.sync.dma_start(gath[:, i * QB:(i + 1) * QB],
                                  ag_out[qh][i * 128:(i + 1) * 128, :])
            for m in range(9):
                esz = min(128, ESH - m * 128)
                dps = dp.tile([128, QB], F32, tag="dp")
                for i in range(NET):
                    nc.tensor.matmul(
                        dps[0:esz, :],
                        lhsT=wdsb[:, i * ESH + m * 128:i * ESH + m * 128 + esz],
                        rhs=gath[:, i * QB:(i + 1) * QB],
                        start=(i == 0), stop=(i == NET - 1),
                    )
                osb = op.tile([128, QB], F32, tag="op")
                nc.scalar.copy(osb[0:esz, :], dps[0:esz, :])
                nc.sync.dma_start(out[m * 128:m * 128 + esz, qh * QB:(qh + 1) * QB],
                                  osb[0:esz, :])

    pers.release()
    dram.release()


def build():
    if "nc" in _CACHE:
        return _CACHE["nc"]
    nc = bacc.Bacc("TRN2", target_bir_lowering=False, debug=False,
                   num_devices=NCORES)
    io = {
        "xT": nc.dram_tensor("xT", [D, L], BF16, kind="ExternalInput").ap(),
        "wqkvT": nc.dram_tensor("wqkvT", [D, DLOC + 128], BF16,
                                kind="ExternalInput").ap(),
        "wdT": nc.dram_tensor("wdT", [DPAD, ESH], BF16, kind="ExternalInput").ap(),
        "cosT": nc.dram_tensor("cosT", [128, L], BF16, kind="ExternalInput").ap(),
        "sinT": nc.dram_tensor("sinT", [128, L], BF16, kind="ExternalInput").ap(),
        "rot": nc.dram_tensor("rot", [128, 128], BF16, kind="ExternalInput").ap(),
        "eye": nc.dram_tensor("eye", [128, 128], BF16, kind="ExternalInput").ap(),
        "masks": nc.dram_tensor("masks", [128, 128], BF16,
                                kind="ExternalInput").ap(),
        "ones64": nc.dram_tensor("ones64", [1, 64], BF16, kind="ExternalInput").ap(),
        "out": nc.dram_tensor("out", [ESH, L], F32, kind="ExternalOutput").ap(),
    }
    with tile.TileContext(nc) as tc:
        _emit(tc, nc, io)
    nc.compile()
    _CACHE["nc"] = nc
    return nc


def make_in_maps(hidden_states, W_qkv, W_dense):
    bf = ml_dtypes.bfloat16
    x = np.asarray(hidden_states, np.float32)
    Wqkv = np.asarray(W_qkv, np.float32)
    Wd = np.asarray(W_dense, np.float32)

    # rope tables, transposed [64, L], replicated to both 64-row halves
    inv = 1.0 / (10000.0 ** (np.arange(0, DKV, 2, dtype=np.float32) / DKV))
    t = np.arange(L, dtype=np.float32)
    freqs = np.outer(t, inv)
    emb = np.concatenate([freqs, freqs], axis=1)          # [L, 64]
    cosT = np.tile(np.cos(emb).T, (2, 1)).astype(bf)      # [128, L]
    sinT = np.tile(np.sin(emb).T, (2, 1)).astype(bf)

    # rotate_half as a matmul: qrot = R1 @ q; lhsT = R1^T; 2-head block diagonal
    R1 = np.zeros((DKV, DKV), np.float32)
    for i in range(32):
        R1[i, i + 32] = -1.0
        R1[i + 32, i] = 1.0
    R2 = np.zeros((128, 128), np.float32)
    R2[:64, :64] = R1
    R2[64:, 64:] = R1
    rot = R2.T.copy().astype(bf)

    eye = np.eye(128, dtype=np.float32).astype(bf)
    ones64 = np.ones((1, 64), np.float32).astype(bf)

    # triangular causal mask for the 128x128 diagonal block
    kk = np.arange(128)[:, None]
    qq = np.arange(128)[None, :]
    masks = (kk <= qq).astype(np.float32).astype(bf)

    # padded dense weights: W_d^T with 64 zero rows appended (pad head)
    wdT_full = np.concatenate([Wd.T, np.zeros((DPAD - D, D), np.float32)], axis=0)
    wdT_full = wdT_full.astype(bf)

    WkvT = Wqkv[H * DKV:].T.astype(bf)                    # [D, 128]

    in_maps = []
    for c in range(NCORES):
        g, j = divmod(c, GSZ)
        h0 = HPC * j
        nh = HPC if j < GSZ - 1 else H - HPC * (GSZ - 1)  # 18,18,18,17
        WqT = np.zeros((D, DLOC), np.float32)
        WqT[:, :nh * DKV] = Wqkv[DKV * h0:DKV * (h0 + nh)].T
        in_maps.append({
            "xT": np.ascontiguousarray(x[g].T).astype(bf),
            "wqkvT": np.concatenate([WqT.astype(bf), WkvT], axis=1),
            "wdT": np.ascontiguousarray(wdT_full[:, ESH * j:ESH * (j + 1)]),
            "cosT": cosT, "sinT": sinT, "rot": rot, "eye": eye,
            "masks": masks, "ones64": ones64,
        })
    return in_maps


def assemble(results):
    out = np.empty((N, L, D), np.float32)
    for c in range(NCORES):
        g, j = divmod(c, GSZ)
        out[g, :, ESH * j:ESH * (j + 1)] = results[c]["out"].T
    return out


def kernel(hidden_states, W_qkv, W_dense):
    nc = build()
    in_maps = make_in_maps(hidden_states, W_qkv, W_dense)
    res = run_bass_kernel_spmd(nc, in_maps, core_ids=list(range(NCORES)))
    return assemble(res.results)


if __name__ == "__main__":
    import reference
    inputs = reference.setup_inputs()
    out = kernel(**{k: np.asarray(v) for k, v in inputs.items()})
    print("out", out.shape, out.dtype)


# revision 21
# speedup vs baseline: 1.2000x; 1.0678x over previous
"""Distributed MQA attention block (N=2, L=1024, D=4544, H=71, Dkv=64) on 8 TRN2 cores.

Sharding: 2 batch groups x 4-way head tensor-parallel.
  core c = 4*g + j: batch g, heads [18j, 18j+18) (core j=3: 17 real heads + 1 zero pad).
Per core: QKV projection (bf16), RoPE (rotation via PE matmul), causal attention in
S^T = K@Q^T orientation (softmax sum fused into the V-matmul via an appended
ones-column), AllGather of attn^T (bf16) within each 4-core group per q-half,
column-sharded dense projection. Host casts inputs to bf16 / pre-transposes, and
assembles the 8 [1136, 1024] f32 output shards.
"""

import sys

if "/opt/trn_rl_repo" not in sys.path:
    sys.path.insert(0, "/opt/trn_rl_repo")

import numpy as np
import ml_dtypes

import concourse.bass as bass
import concourse.bacc as bacc
import concourse.mybir as mybir
import concourse.tile as tile
from concourse.bass_utils import run_bass_kernel_spmd

BF16 = mybir.dt.bfloat16
F32 = mybir.dt.float32
AF = mybir.ActivationFunctionType

N, L, D = 2, 1024, 4544
H, DKV = 71, 64
NCORES, GSZ = 8, 4
HPC = 18                 # heads per core (last core of each group: 17 real + 1 pad)
DLOC = HPC * DKV         # 1152
DPAD = GSZ * DLOC        # 4608 = 36 * 128
ESH = D // GSZ           # 1136 output-column shard
NET = 36                 # e-contraction tiles over D=4544 (35 x 128 + 1 x 64)
NMT = DLOC // 128        # 9 m-tiles of Q^T rows (2 heads each)
QB = 512                 # q-block (half of L)
SCALE = 1.0 / np.sqrt(DKV)
REPLICA_GROUPS = [[0, 1, 2, 3], [4, 5, 6, 7]]

_CACHE = {}


def _esz(i):
    return 128 if i < NET - 1 else 64


def _emit(tc, nc, io):
    xT, wqkvT, wdT, cosT, sinT, rot, eye, masks, ones64, out = (
        io["xT"], io["wqkvT"], io["wdT"], io["cosT"], io["sinT"], io["rot"],
        io["eye"], io["masks"], io["ones64"], io["out"],
    )

    # ---- persistent SBUF (live through the whole kernel) ----
    pers = tc.alloc_tile_pool(name="pers", bufs=1)
    qsb = pers.tile([128, NMT * 1024], BF16, tag="qsb")    # roped Q^T, 2 heads/tile
    ksb = pers.tile([128, 1024], BF16, tag="ksb")          # roped K^T, dup in both halves
    vsb = pers.tile([128, 8 * 65], BF16, tag="vsb")        # V [tok,64]+ones col, 8 chunks
    cossb = pers.tile([128, 1024], BF16, tag="cossb")
    sinsb = pers.tile([128, 1024], BF16, tag="sinsb")
    rotsb = pers.tile([128, 128], BF16, tag="rotsb")
    eyesb = pers.tile([128, 128], BF16, tag="eyesb")
    masksb = pers.tile([128, 128], BF16, tag="masksb")
    onesb = pers.tile([1, 64], BF16, tag="onesb")

    nc.sync.dma_start(cossb[:, :], cosT[:, :])
    nc.sync.dma_start(sinsb[:, :], sinT[:, :])
    nc.sync.dma_start(rotsb[:, :], rot[:, :])
    nc.sync.dma_start(eyesb[:, :], eye[:, :])
    nc.sync.dma_start(masksb[:, :], masks[:, :])
    nc.sync.dma_start(onesb[:, :], ones64[:, :])
    nc.vector.memset(vsb[:, :], 1.0)  # ones column survives the V copies below

    # ---- DRAM bounce buffers for the per-q-half AllGather ----
    dram = tc.alloc_tile_pool(name="dram", bufs=1, space="DRAM")
    ag_in = [dram.tile([DLOC, QB], BF16, tag=f"agin{q}", name=f"agin{q}")
             for q in range(2)]
    ag_out = [dram.tile([DPAD, QB], BF16, tag=f"agout{q}", name=f"agout{q}")
              for q in range(2)]

    # ================= Phase A: QKV projection + RoPE =================
    with (
        tc.tile_pool(name="xp", bufs=1) as xp,
        tc.tile_pool(name="wqp", bufs=2) as wqp,
        tc.tile_pool(name="ra", bufs=2) as ra,
        tc.tile_pool(name="rt", bufs=2) as rt,
        tc.tile_pool(name="vt", bufs=1) as vt,
        tc.tile_pool(name="ps2", bufs=3, space="PSUM") as ps2,
    ):
        xsb = xp.tile([128, NET * 1024], BF16, tag="xsb")
        for i in range(NET):
            p = _esz(i)
            nc.sync.dma_start(xsb[0:p, i * 1024:(i + 1) * 1024],
                              xT[i * 128:i * 128 + p, :])

        def load_w_cols(dst, c0, cw):
            # wqkvT[:, c0:c0+cw] -> dst [128, NET*cw] (tile i at cols i*cw)
            nc.sync.dma_start(
                dst[:, 0:(NET - 1) * cw].rearrange("p (t c) -> p t c", c=cw),
                wqkvT[0:(NET - 1) * 128, c0:c0 + cw].rearrange(
                    "(t p) c -> p t c", p=128),
            )
            nc.sync.dma_start(dst[0:64, (NET - 1) * cw:NET * cw],
                              wqkvT[(NET - 1) * 128:D, c0:c0 + cw])

        # ---- K/V projections (shared KV head), fused: out rows 0:64 = K^T,
        # rows 64:128 = V^T
        wkv = wqp.tile([128, NET * 128], BF16, tag="wm")
        load_w_cols(wkv, DLOC, 128)

        kvps = ps2.tile([128, 1024], F32, tag="ps2")
        for i in range(NET):
            p = _esz(i)
            for q in range(2):
                nc.tensor.matmul(
                    kvps[:, q * QB:(q + 1) * QB],
                    lhsT=wkv[0:p, i * 128:(i + 1) * 128],
                    rhs=xsb[0:p, i * 1024 + q * QB:i * 1024 + (q + 1) * QB],
                    start=(i == 0), stop=(i == NET - 1),
                )
        kvraw = ra.tile([128, 1024], BF16, tag="ra")
        nc.scalar.copy(kvraw[:, :], kvps[:, :])
        # rope K (rows 0:64), then duplicate into rows 64:128 via DMA
        krot = ps2.tile([128, 1024], F32, tag="ps2")
        for q in range(2):
            nc.tensor.matmul(krot[0:64, q * QB:(q + 1) * QB],
                             lhsT=rotsb[0:64, 0:64],
                             rhs=kvraw[0:64, q * QB:(q + 1) * QB],
                             start=True, stop=True)
        for q in range(2):
            s = slice(q * QB, (q + 1) * QB)
            t1 = rt.tile([128, QB], F32, tag="t1")
            t2 = rt.tile([128, QB], F32, tag="t2")
            nc.vector.tensor_mul(t1[0:64, :], kvraw[0:64, s], cossb[0:64, s])
            nc.vector.tensor_mul(t2[0:64, :], krot[0:64, s], sinsb[0:64, s])
            nc.vector.tensor_add(ksb[0:64, s], t1[0:64, :], t2[0:64, :])
        nc.sync.dma_start(ksb[64:128, :], ksb[0:64, :])

        # V^T (kvraw rows 64:128) -> transpose to V [tok, 64] chunks in vsb
        for t8 in range(8):
            vtp = ps2.tile([128, 2048], BF16, tag="ps2")
            nc.tensor.transpose(vtp[0:128, 0:64],
                                kvraw[64:128, t8 * 128:(t8 + 1) * 128],
                                eyesb[64:128, 64:128])
            nc.scalar.copy(vsb[:, t8 * 65:t8 * 65 + 64], vtp[0:128, 0:64])

        # ---- Q projection + RoPE, per m-tile (2 heads each) ----
        for m in range(NMT):
            wqm = wqp.tile([128, NET * 128], BF16, tag="wm")
            load_w_cols(wqm, m * 128, 128)
            qps = ps2.tile([128, 1024], F32, tag="ps2")
            for i in range(NET):
                p = _esz(i)
                for q in range(2):
                    nc.tensor.matmul(
                        qps[:, q * QB:(q + 1) * QB],
                        lhsT=wqm[0:p, i * 128:(i + 1) * 128],
                        rhs=xsb[0:p, i * 1024 + q * QB:i * 1024 + (q + 1) * QB],
                        start=(i == 0), stop=(i == NET - 1),
                    )
            qraw = ra.tile([128, 1024], BF16, tag="ra")
            nc.scalar.copy(qraw[:, :], qps[:, :])
            qrot = ps2.tile([128, 1024], F32, tag="ps2")
            for q in range(2):
                nc.tensor.matmul(qrot[:, q * QB:(q + 1) * QB],
                                 lhsT=rotsb[:, :],
                                 rhs=qraw[:, q * QB:(q + 1) * QB],
                                 start=True, stop=True)
            for q in range(2):
                s = slice(q * QB, (q + 1) * QB)
                t1 = rt.tile([128, QB], F32, tag="t1")
                t2 = rt.tile([128, QB], F32, tag="t2")
                nc.vector.tensor_mul(t1[:, :], qraw[:, s], cossb[:, s])
                nc.vector.tensor_mul(t2[:, :], qrot[:, s], sinsb[:, s])
                nc.vector.tensor_add(qsb[:, m * 1024 + q * QB:m * 1024 + (q + 1) * QB],
                                     t1[:, :], t2[:, :])

    # ================= Phase B: attention + AllGather + dense =================
    with (
        tc.tile_pool(name="ex", bufs=18) as ex,
        tc.tile_pool(name="at", bufs=4) as at,
        tc.tile_pool(name="rb", bufs=2) as rb,
        tc.tile_pool(name="rp", bufs=2) as rp,
        tc.tile_pool(name="rpf", bufs=2) as rpf,
        tc.tile_pool(name="wdp", bufs=1) as wdp,
        tc.tile_pool(name="gp", bufs=1) as gp,
        tc.tile_pool(name="op", bufs=3) as op,
        tc.tile_pool(name="sc", bufs=3, space="PSUM") as sc,
        tc.tile_pool(name="ac", bufs=4, space="PSUM") as ac,
        tc.tile_pool(name="dp", bufs=1, space="PSUM") as dp,
    ):
        # dense weights resident; DMAs run in the background during attention
        wdsb = wdp.tile([128, NET * ESH], BF16, tag="wdsb")
        for i in range(NET):
            nc.sync.dma_start(wdsb[:, i * ESH:(i + 1) * ESH],
                              wdT[i * 128:(i + 1) * 128, :])

        for qh in range(2):
            nkt = 4 * qh + 4
            for hp in range(NMT):
                # --- S burst: scores + exp for both heads of the pair; the
                # diagonal k-tile of q-block qh is k-tile 4*qh+j: columns
                # below 128*j are fully masked (skip compute, memset 0),
                # the [128j, 128j+128) block gets the triangular mask.
                ess = [[None] * nkt for _ in range(2)]
                for kt in range(nkt):
                    var = kt - 4 * qh
                    c0 = 128 * var if var > 0 else 0
                    for par in range(2):  # interleave: even/odd use PE row
                        off = 64 * par    # groups 0:64 / 64:128 concurrently
                        scp = sc.tile([128, QB], F32, tag="sc")
                        nc.tensor.matmul(
                            scp[:, c0:QB],
                            lhsT=ksb[off:off + 64, kt * 128:(kt + 1) * 128],
                            rhs=qsb[off:off + 64,
                                    hp * 1024 + qh * QB + c0:hp * 1024 + (qh + 1) * QB],
                            start=True, stop=True,
                        )
                        es = ex.tile([128, QB], BF16, tag="ex")
                        if c0 > 0:
                            nc.vector.memset(es[:, 0:c0], 0.0)
                        nc.scalar.activation(es[:, c0:QB], scp[:, c0:QB],
                                             AF.Exp, scale=SCALE)
                        if var >= 0:  # triangular mask on the diagonal block
                            nc.vector.tensor_mul(
                                es[:, c0:c0 + 128], es[:, c0:c0 + 128],
                                masksb[:, 0:128])
                        ess[par][kt] = es
                # --- V burst: back-to-back accumulating matmuls on PE
                accs = []
                for par in range(2):
                    h = 2 * hp + par
                    acc = ac.tile([128, QB], F32, tag="ac")
                    for kt in range(nkt):
                        nc.tensor.matmul(
                            acc[0:65, :],
                            lhsT=vsb[:, kt * 65:(kt + 1) * 65],
                            rhs=ess[par][kt][:, :],
                            start=(kt == 0), stop=(kt == nkt - 1),
                        )
                    accs.append((h, acc))
                # --- normalize: 1/sum (row 64), broadcast via K=1 matmul
                for h, acc in accs:
                    stg = rpf.tile([1, QB], F32, tag="stg")
                    nc.scalar.copy(stg[:, :], acc[64:65, :])
                    rf = rpf.tile([1, QB], F32, tag="rpf")
                    nc.vector.reciprocal_approx_fast(rf[:, :], stg[:, :])
                    r = rp.tile([1, QB], BF16, tag="rp")
                    with nc.allow_low_precision(reason="softmax 1/sum in bf16"):
                        nc.vector.tensor_copy(r[:, :], rf[:, :])
                    rbp = sc.tile([128, QB], F32, tag="sc")
                    nc.tensor.matmul(rbp[0:64, :], lhsT=onesb[0:1, 0:64],
                                     rhs=r[0:1, :], start=True, stop=True)
                    rbs = rb.tile([64, QB], BF16, tag="rb")
                    nc.vector.tensor_copy(rbs[:, :], rbp[0:64, :])
                    asb = at.tile([64, QB], BF16, tag="at")
                    nc.vector.tensor_mul(asb[:, :], acc[0:64, :], rbs[:, :])
                    nc.sync.dma_start(ag_in[qh][64 * h:64 * (h + 1), :], asb[:, :])

            nc.gpsimd.collective_compute(
                "AllGather",
                mybir.AluOpType.bypass,
                ins=[ag_in[qh].opt()],
                outs=[ag_out[qh].opt()],
                replica_groups=REPLICA_GROUPS,
            )

        # ---- dense: out^T[e_shard, q] = W_d^T[dpad, e].T @ attn^T[dpad, q] ----
        for qh in range(2):
            gath = gp.tile([128, NET * QB], BF16, tag="gath")
            for i in range(NET):
                nc.sync.dma_start(gath[:, i * QB:(i + 1) * QB],
                                  ag_out[qh][i * 128:(i + 1) * 128, :])
            for m in range(9):
                esz = min(128, ESH - m * 128)
                dps = dp.tile([128, QB], F32, tag="dp")
                for i in range(NET):
                    nc.tensor.matmul(
                        dps[0:esz, :],
                        lhsT=wdsb[:, i * ESH + m * 128:i * ESH + m * 128 + esz],
                        rhs=gath[:, i * QB:(i + 1) * QB],
                        start=(i == 0), stop=(i == NET - 1),
                    )
                osb = op.tile([128, QB], F32, tag="op")
                nc.scalar.copy(osb[0:esz, :], dps[0:esz, :])
                nc.sync.dma_start(out[m * 128:m * 128 + esz, qh * QB:(qh + 1) * QB],
                                  osb[0:esz, :])

    pers.release()
    dram.release()


def build():
    if "nc" in _CACHE:
        return _CACHE["nc"]
    nc = bacc.Bacc("TRN2", target_bir_lowering=False, debug=False,
                   num_devices=NCORES)
    io = {
        "xT": nc.dram_tensor("xT", [D, L], BF16, kind="ExternalInput").ap(),
        "wqkvT": nc.dram_tensor("wqkvT", [D, DLOC + 128], BF16,
                                kind="ExternalInput").ap(),
        "wdT": nc.dram_tensor("wdT", [DPAD, ESH], BF16, kind="ExternalInput").ap(),
        "cosT": nc.dram_tensor("cosT", [128, L], BF16, kind="ExternalInput").ap(),
        "sinT": nc.dram_tensor("sinT", [128, L], BF16, kind="ExternalInput").ap(),
        "rot": nc.dram_tensor("rot", [128, 128], BF16, kind="ExternalInput").ap(),
        "eye": nc.dram_tensor("eye", [128, 128], BF16, kind="ExternalInput").ap(),
        "masks": nc.dram_tensor("masks", [128, 128], BF16,
                                kind="ExternalInput").ap(),
        "ones64": nc.dram_tensor("ones64", [1, 64], BF16, kind="ExternalInput").ap(),
        "out": nc.dram_tensor("out", [ESH, L], F32, kind="ExternalOutput").ap(),
    }
    with tile.TileContext(nc) as tc:
        _emit(tc, nc, io)
    nc.compile()
    _CACHE["nc"] = nc
    return nc


def make_in_maps(hidden_states, W_qkv, W_dense):
    bf = ml_dtypes.bfloat16
    x = np.asarray(hidden_states, np.float32)
    Wqkv = np.asarray(W_qkv, np.float32)
    Wd = np.asarray(W_dense, np.float32)

    # rope tables, transposed [64, L], replicated to both 64-row halves
    inv = 1.0 / (10000.0 ** (np.arange(0, DKV, 2, dtype=np.float32) / DKV))
    t = np.arange(L, dtype=np.float32)
    freqs = np.outer(t, inv)
    emb = np.concatenate([freqs, freqs], axis=1)          # [L, 64]
    cosT = np.tile(np.cos(emb).T, (2, 1)).astype(bf)      # [128, L]
    sinT = np.tile(np.sin(emb).T, (2, 1)).astype(bf)

    # rotate_half as a matmul: qrot = R1 @ q; lhsT = R1^T; 2-head block diagonal
    R1 = np.zeros((DKV, DKV), np.float32)
    for i in range(32):
        R1[i, i + 32] = -1.0
        R1[i + 32, i] = 1.0
    R2 = np.zeros((128, 128), np.float32)
    R2[:64, :64] = R1
    R2[64:, 64:] = R1
    rot = R2.T.copy().astype(bf)

    eye = np.eye(128, dtype=np.float32).astype(bf)
    ones64 = np.ones((1, 64), np.float32).astype(bf)

    # triangular causal mask for the 128x128 diagonal block
    kk = np.arange(128)[:, None]
    qq = np.arange(128)[None, :]
    masks = (kk <= qq).astype(np.float32).astype(bf)

    # padded dense weights: W_d^T with 64 zero rows appended (pad head)
    wdT_full = np.concatenate([Wd.T, np.zeros((DPAD - D, D), np.float32)], axis=0)
    wdT_full = wdT_full.astype(bf)

    WkvT = Wqkv[H * DKV:].T.astype(bf)                    # [D, 128]

    in_maps = []
    for c in range(NCORES):
        g, j = divmod(c, GSZ)
        h0 = HPC * j
        nh = HPC if j < GSZ - 1 else H - HPC * (GSZ - 1)  # 18,18,18,17
        WqT = np.zeros((D, DLOC), np.float32)
        WqT[:, :nh * DKV] = Wqkv[DKV * h0:DKV * (h0 + nh)].T
        in_maps.append({
            "xT": np.ascontiguousarray(x[g].T).astype(bf),
            "wqkvT": np.concatenate([WqT.astype(bf), WkvT], axis=1),
            "wdT": np.ascontiguousarray(wdT_full[:, ESH * j:ESH * (j + 1)]),
            "cosT": cosT, "sinT": sinT, "rot": rot, "eye": eye,
            "masks": masks, "ones64": ones64,
        })
    return in_maps


def assemble(results):
    out = np.empty((N, L, D), np.float32)
    for c in range(NCORES):
        g, j = divmod(c, GSZ)
        out[g, :, ESH * j:ESH * (j + 1)] = results[c]["out"].T
    return out


def kernel(hidden_states, W_qkv, W_dense):
    nc = build()
    in_maps = make_in_maps(hidden_states, W_qkv, W_dense)
    res = run_bass_kernel_spmd(nc, in_maps, core_ids=list(range(NCORES)))
    return assemble(res.results)


if __name__ == "__main__":
    import reference
    inputs = reference.setup_inputs()
    out = kernel(**{k: np.asarray(v) for k, v in inputs.items()})
    print("out", out.shape, out.dtype)
